# revision 9
# baseline (speedup 1.0000x reference)
"""DGMC graph-matching kernel for 8 Trainium2 NeuronCores.

Strategy (data-parallel over the B=8 graph dimension, one graph per core):

 * Every matmul is phrased in "transposed" (feature-major) form so that no
   on-device transposes are needed; the host supplies pre-transposed copies
   of the small inputs packed into a few large [128, X] blobs (one DMA each).
 * Edge lists are converted on the host into dense per-graph count matrices
   A^T[n, d] (#edges n->d), so GNN sum-aggregation becomes plain matmuls.
 * The expensive [Ns, Nt, R] pairwise-difference MLP never materializes:
   relu((o_s[s]-o_t[t]) @ Wm1 + bm1) @ Wm2 distributes over the difference,
   so with A_s = o_s@Wm1, A_t = o_t@Wm1 ([512, 32] each) the update is
       d[s, t] = sum_r Wm2[r] * relu(A_s[s, r] - A_t[t, r] + bm1[r]) + bm2.
   Pairwise tiles X[(j, r), t] = relu(A_s[4*jj+j, r] - A_t[t, r] + bm1[r])
   (partition = 4 s-values x 32 channels) are generated by fused
   add+relu ops split across the Vector and Scalar engines, and reduced over
   the 32 channels by PE matmuls with shifted block-diagonal Wm2 weights that
   accumulate directly into a persistent PSUM copy of S_hat.
 * Softmax normalization is folded into a row-scaling of r_s (the normalized
   attention matrix is never materialized).
 * T = 1/dist is pure input preprocessing -> computed on host, DMA'd in.
"""

import numpy as np
from functools import lru_cache

B, N, C, R = 8, 512, 64, 32
NT = N // 128  # 4 partition-tiles per 512
NCORES = 8

# matmul sites that run in float32 instead of float32r (higher precision,
# 4x slower per row).  float32r is ~1 cycle/row at N>=512.
F32_SITES = {"shat0", "psi1_agg", "psi1_h"}

# fraction of X tiles produced on the scalar engine (rest on vector engine)
ACT_EVERY = 3  # k % ACT_EVERY == 2 -> ACT
STEPS = 2  # debug knob


# --------------------------------------------------------------------------
# host-side packing
# --------------------------------------------------------------------------

# wblob column layout (partition rows x free cols), all f32:
#   W1s   [64, 64]   @ (0,    0)
#   W1n   [64, 64]   @ (0,   64)
#   W2s   [32, 32]   @ (0,  128)
#   W2n   [32, 32]   @ (0,  160)
#   negWm1x4 [32,128] @ (0,  192)
#   Wm1bank [32,512] @ (0,  320)
#   b1c   [64, 1]    @ (0,  832)
#   b2c   [32, 1]    @ (0,  833)
#   bm1x4 [128, 1]   @ (0,  834)
#   ones1 [1, 128]   @ (0,  840)
#   bm2row [1, 512]  @ (0,  968)
WBLOB_F = 1480


def _build_wblob(W1_self, W1_nbr, b1, W2_self, W2_nbr, b2, Wm1, bm1, Wm2, bm2):
    wb = np.zeros((128, WBLOB_F), np.float32)
    wb[0:64, 0:64] = W1_self
    wb[0:64, 64:128] = W1_nbr
    wb[0:32, 128:160] = W2_self
    wb[0:32, 160:192] = W2_nbr
    wb[0:32, 192:320] = np.tile(-Wm1, (1, 4))
    for j in range(4):
        wb[0:32, 320 + 128 * j + 32 * j: 320 + 128 * j + 32 * (j + 1)] = Wm1
    wb[0:64, 832] = b1
    wb[0:32, 833] = b2
    wb[0:128, 834] = np.tile(bm1, 4)
    wb[0:1, 840:968] = 1.0
    wb[0:128, 836] = -1.0 / 512.0
    wb[0:1, 968:1480] = float(bm2[0])
    return wb


def _build_wbank(Wm2):
    # 32 shifted block-diagonal lhsT blocks of [128, 128]:
    #   wbank[32*j + r, 128*k + m] = Wm2[r] iff m == 4*k + j
    w2 = Wm2[:, 0].astype(np.float32)
    bank = np.zeros((128, 128 * 32), np.float32)
    for k in range(32):
        for j in range(4):
            bank[32 * j: 32 * (j + 1), 128 * k + 4 * k + j] = w2
    return bank


def _tile4(x):  # [512, F] -> [128, 4*F] (partition-tiles side by side)
    F = x.shape[1]
    out = np.empty((128, 4 * F), np.float32)
    for b in range(4):
        out[:, F * b: F * (b + 1)] = x[128 * b: 128 * (b + 1)]
    return out


def _adjT(edge_index, g):
    # A^T[n, d] = #edges with src n -> dst d  (graph-local indices)
    src = np.asarray(edge_index[0], np.int64)
    dst = np.asarray(edge_index[1], np.int64)
    lo, hi = g * N, (g + 1) * N
    m = (src >= lo) & (src < hi)
    a = np.zeros((N, N), np.float32)
    np.add.at(a, (src[m] - lo, dst[m] - lo), 1.0)
    return a


def _host_random(num_steps):
    # Must exactly mirror the reference: jax.random on the *default* device
    # (the PRNG impl here is rbg, whose bits are platform-dependent).
    import jax

    rkey = jax.random.key(42)
    return [
        np.asarray(
            jax.random.normal(jax.random.fold_in(rkey, step), (B, N, R), "float32")
        )
        for step in range(num_steps)
    ]


# --------------------------------------------------------------------------
# the Bass program (identical for all cores)
# --------------------------------------------------------------------------


@lru_cache(maxsize=1)
def _build_nc():
    import concourse.bacc as bacc
    import concourse.bass as bass
    import concourse.tile as tile
    from concourse import mybir

    dt = mybir.dt
    f32 = dt.float32
    f32r = dt.float32r
    AF = mybir.ActivationFunctionType
    ALU = mybir.AluOpType
    AX = mybir.AxisListType

    nc = bacc.Bacc("TRN2", target_bir_lowering=False, debug=False)

    wblob_d = nc.dram_tensor("wblob", [128, WBLOB_F], f32, kind="ExternalInput")
    xblob_d = nc.dram_tensor("xblob", [128, 1536], f32, kind="ExternalInput")
    ablob_d = nc.dram_tensor("ablob", [128, 4096], f32, kind="ExternalInput")
    rblob_d = nc.dram_tensor("rblob", [128, 1280], f32, kind="ExternalInput")
    wbank_d = nc.dram_tensor("wbank", [128, 4096], f32, kind="ExternalInput")
    tblob_d = nc.dram_tensor("tblob", [128, 2048], f32, kind="ExternalInput")

    shat_d = nc.dram_tensor("shat_out", [N, N], f32, kind="ExternalOutput")
    slp_d = nc.dram_tensor("slp_out", [N, N], f32, kind="ExternalOutput")
    soft_d = nc.dram_tensor("soft_out", [N], f32, kind="ExternalOutput")

    def mm(out, lhsT, rhs, site, **kw):
        t = f32 if site in F32_SITES else f32r
        nc.tensor.matmul(out, lhsT.bitcast(t), rhs.bitcast(t), **kw)

    with tile.TileContext(nc) as tc:
        with (
            tc.tile_pool(name="cp", bufs=1) as cp,
            tc.tile_pool(name="xp", bufs=6) as xp,
            tc.tile_pool(name="op", bufs=2) as op_,
            tc.tile_pool(name="shp", bufs=1, space=bass.MemorySpace.PSUM) as shp,
            tc.tile_pool(name="wp", bufs=3, space=bass.MemorySpace.PSUM) as wp,
        ):
            # ---------------- persistent SBUF tiles + input DMAs
            wblob = cp.tile([128, WBLOB_F], f32, tag="wblob", name="wblob")
            wblob_r = cp.tile([128, WBLOB_F], f32r, tag="wblob_r", name="wblob_r")
            xblob = cp.tile([128, 1536], f32, tag="xblob", name="xblob")
            ablob_f = cp.tile([128, 4096], f32, tag="ablob_f", name="ablob_f")
            ablob = cp.tile([128, 4096], f32r, tag="ablob", name="ablob")
            rblob = cp.tile([128, 1280], f32r, tag="rblob", name="rblob")
            wbank = cp.tile([128, 4096], f32r, tag="wbank", name="wbank")
            tblob = cp.tile([128, 2048], f32, tag="tblob", name="tblob")
            nc.sync.dma_start(xblob[:], xblob_d[:])
            nc.sync.dma_start(wblob[:], wblob_d[:])
            nc.sync.dma_start(wblob_r[:], wblob_d.ap().bitcast(f32r))
            nc.sync.dma_start(ablob_f[:], ablob_d[:])
            nc.sync.dma_start(ablob[:], ablob_d.ap().bitcast(f32r))
            nc.sync.dma_start(rblob[:], rblob_d.ap().bitcast(f32r))
            nc.sync.dma_start(wbank[:], wbank_d.ap().bitcast(f32r))
            nc.sync.dma_start(tblob[:], tblob_d[:])

            W1s = wblob[0:64, 0:64]
            W1n = wblob[0:64, 64:128]
            W2s = wblob_r[0:32, 128:160]
            W2n = wblob_r[0:32, 160:192]
            negWm1x4 = wblob_r[0:32, 192:320]
            Wm1bank = wblob_r[0:32, 320:832]
            b1c = wblob[0:64, 832:833]
            b2c = wblob[0:32, 833:834]
            bm1x4 = wblob[0:128, 834:835]
            zcol_c = wblob[0:128, 835:836]
            neginvN = wblob[0:128, 836:837]
            ones1 = wblob_r[0:1, 840:968]
            bm2row = wblob_r[0:1, 968:1480]

            xs4 = xblob[:, 0:256]        # [128, 4*64] natural tiles
            xt4 = xblob[:, 256:512]
            xsT = xblob[0:64, 512:1024]  # [64, 512]
            xtT = xblob[0:64, 1024:1536]
            AsT4 = ablob[:, 0:2048]      # [128, 4*512]
            AtT4 = ablob[:, 2048:4096]
            rs4 = [rblob[:, 0:128], rblob[:, 128:256]]       # [128, 4*32] per step
            rsT = [rblob[0:32, 256:768], rblob[0:32, 768:1280]]  # [32, 512]

            # persistent PSUM: S_hat, one bank per 128-row tile
            shat = [shp.tile([128, N], f32, tag=f"shat{sb}", name=f"shat{sb}") for sb in range(NT)]

            # ---------------- psi_1 + S_hat_0
            hT = {}
            for side, x4, xT, AT4 in (
                ("s", xs4, xsT, ablob_f[:, 0:2048]),
                ("t", xt4, xtT, ablob_f[:, 2048:4096]),
            ):
                agg_ps = wp.tile([64, N], f32, tag="w", name="w")
                for nb in range(NT):
                    mm(
                        agg_ps[:],
                        x4[:, 64 * nb: 64 * (nb + 1)],
                        AT4[:, N * nb: N * (nb + 1)],
                        "psi1_agg",
                        start=(nb == 0),
                        stop=(nb == NT - 1),
                    )
                agg_sb = cp.tile([64, N], f32, tag=f"agg1{side}", name=f"agg1{side}")
                nc.scalar.activation(agg_sb[:], agg_ps[:], AF.Identity, bias=zcol_c[0:agg_sb.shape[0]])
                h_ps = wp.tile([64, N], f32, tag="w", name="w")
                mm(h_ps[:], W1s, xT, "psi1_h", start=True, stop=False)
                mm(h_ps[:], W1n, agg_sb[:], "psi1_h", start=False, stop=True)
                hT[side] = cp.tile([64, N], f32, tag=f"hT{side}", name=f"hT{side}")
                nc.scalar.activation(hT[side][:], h_ps[:], AF.Relu, bias=b1c)

            for sb in range(NT):
                mm(
                    shat[sb][:],
                    hT["s"][:, 128 * sb: 128 * (sb + 1)],
                    hT["t"][:],
                    "shat0",
                    start=True,
                    stop=(STEPS == 0),
                    skip_group_check=True,
                )

            # ---------------- consensus steps
            for step in range(STEPS):
                # softmax pieces: E = exp(shat - rowmax), Z = row sums
                negmax = cp.tile([128, NT], f32, tag="negmax", name="negmax")
                zcol = cp.tile([128, NT], f32, tag="zcol", name="zcol")
                E4 = cp.tile([128, NT * N], f32r, tag="E4", name="E4")
                for sb in range(NT):
                    nc.vector.tensor_reduce(
                        negmax[:, sb: sb + 1],
                        shat[sb][:],
                        AX.X,
                        ALU.max,
                        negate=True,
                    )
                    nc.scalar.activation(
                        E4[:, N * sb: N * (sb + 1)],
                        shat[sb][:],
                        AF.Exp,
                        bias=negmax[:, sb: sb + 1],
                        accum_out=zcol[:, sb: sb + 1],
                    )
                rcp = cp.tile([128, NT], f32, tag="rcp", name="rcp")
                nc.vector.reciprocal(rcp[:], zcol[:])
                # scaled r_s rows (folds the softmax normalization)
                rs_sc = cp.tile([128, NT * R], f32r, tag="rs_sc", name="rs_sc")
                for sb in range(NT):
                    nc.vector.tensor_scalar_mul(
                        rs_sc[:, R * sb: R * (sb + 1)],
                        rs4[step][:, R * sb: R * (sb + 1)].bitcast(f32),
                        rcp[:, sb: sb + 1],
                    )

                # r_t^T [32, 512] = (r_s/Z)^T @ E
                rtT_ps = wp.tile([32, N], f32, tag="w", name="w")
                for sb in range(NT):
                    mm(
                        rtT_ps[:],
                        rs_sc[:, R * sb: R * (sb + 1)],
                        E4[:, N * sb: N * (sb + 1)],
                        "rtT",
                        start=(sb == 0),
                        stop=(sb == NT - 1),
                    )
                rtT = cp.tile([32, N], f32r, tag="rtT", name="rtT")
                nc.scalar.activation(rtT[:], rtT_ps[:], AF.Identity, bias=zcol_c[0:32])

                # r_t natural [512, 32] as 4 tiles in [128, 4*32]
                rt4 = cp.tile([128, NT * R], f32r, tag="rt4", name="rt4")
                for tb in range(NT):
                    rt_ps = wp.tile([128, R], f32, tag="w", name="w")
                    for sb in range(NT):
                        mm(
                            rt_ps[:],
                            E4[:, N * sb + 128 * tb: N * sb + 128 * (tb + 1)],
                            rs_sc[:, R * sb: R * (sb + 1)],
                            "rtnat",
                            start=(sb == 0),
                            stop=(sb == NT - 1),
                        )
                    nc.vector.tensor_copy(
                        rt4[:, R * tb: R * (tb + 1)], rt_ps[:]
                    )

                # psi_2 GNN, both sides -> o^T [32, 512]
                oT = {}
                for side, rnat, rT, AT4 in (
                    ("s", rs4[step], rsT[step], AsT4),
                    ("t", rt4, rtT, AtT4),
                ):
                    agg_ps = wp.tile([32, N], f32, tag="w", name="w")
                    for nb in range(NT):
                        mm(
                            agg_ps[:],
                            rnat[:, R * nb: R * (nb + 1)],
                            AT4[:, N * nb: N * (nb + 1)],
                            "agg2",
                            start=(nb == 0),
                            stop=(nb == NT - 1),
                        )
                    agg_sb = cp.tile([32, N], f32r, tag=f"agg2{side}", name=f"agg2{side}")
                    nc.scalar.activation(agg_sb[:], agg_ps[:], AF.Identity, bias=zcol_c[0:agg_sb.shape[0]])
                    o_ps = wp.tile([32, N], f32, tag="w", name="w")
                    mm(o_ps[:], W2s, rT[:] if side == "s" else rT[:], "o2",
                       start=True, stop=False)
                    mm(o_ps[:], W2n, agg_sb[:], "o2", start=False, stop=True)
                    oT[side] = cp.tile([32, N], f32r, tag=f"oT{side}", name=f"oT{side}")
                    nc.scalar.activation(oT[side][:], o_ps[:], AF.Relu, bias=b2c)

                # A_mlp: repneg = -(o_t @ Wm1)^T replicated to 128 partitions
                repneg_ps = wp.tile([128, N], f32, tag="w", name="w")
                mm(repneg_ps[:], negWm1x4, oT["t"][:], "amlp",
                   start=True, stop=True)
                repneg = cp.tile([128, N], f32, tag="repneg", name="repneg")
                nc.scalar.activation(repneg[:], repneg_ps[:], AF.Identity, bias=zcol_c)

                # biasb[(j, r), jj] = (o_s @ Wm1)[4*jj + j, r] + bm1[r]
                biasb_ps = wp.tile([128, 128], f32, tag="w", name="w")
                for j in range(4):
                    mm(
                        biasb_ps[:],
                        Wm1bank[:, 128 * j: 128 * (j + 1)],
                        oT["s"][:, j::4],
                        "amlp",
                        start=(j == 0),
                        stop=(j == 3),
                    )
                biasb = cp.tile([128, 128], f32, tag="biasb", name="biasb")
                nc.scalar.activation(biasb[:], biasb_ps[:], AF.Identity, bias=bm1x4)

                # pairwise X tiles + PE channel-reduction into S_hat psum
                for sb in range(NT):
                    for k in range(32):
                        jj = 32 * sb + k
                        X = xp.tile([128, N], f32r, tag="x", name="x")
                        if k % ACT_EVERY == ACT_EVERY - 1:
                            nc.scalar.activation(
                                X[:],
                                repneg[:],
                                AF.Relu,
                                bias=biasb[:, jj: jj + 1],
                            )
                        else:
                            nc.vector.tensor_scalar(
                                X[:],
                                repneg[:],
                                biasb[:, jj: jj + 1],
                                0.0,
                                ALU.add,
                                ALU.max,
                            )
                        mm(
                            shat[sb][:],
                            wbank[:, 128 * k: 128 * (k + 1)],
                            X[:],
                            "xw",
                            start=False,
                            stop=False,
                            skip_group_check=True,
                        )
                    mm(
                        shat[sb][:],
                        ones1,
                        bm2row,
                        "bm2",
                        start=False,
                        stop=(step == STEPS - 1),
                        skip_group_check=True,
                    )

            # ---------------- final: S_LP = softmax(S_hat + T), std
            negmax2 = cp.tile([128, NT], f32, tag="negmax2", name="negmax2")
            z2 = cp.tile([128, NT], f32, tag="z2", name="z2")
            Eo4 = cp.tile([128, NT * N], f32, tag="Eo4", name="Eo4")
            msum = cp.tile([128, NT], f32, tag="msum", name="msum")
            for sb in range(NT):
                Zt = op_.tile([128, N], f32, tag="zt", name="zt")
                nc.vector.tensor_tensor(
                    Zt[:], shat[sb][:], tblob[:, N * sb: N * (sb + 1)], ALU.add
                )
                nc.vector.tensor_reduce(
                    negmax2[:, sb: sb + 1], Zt[:], AX.X, ALU.max, negate=True
                )
                nc.scalar.activation(
                    Eo4[:, N * sb: N * (sb + 1)],
                    Zt[:],
                    AF.Exp,
                    bias=negmax2[:, sb: sb + 1],
                    accum_out=z2[:, sb: sb + 1],
                )
                sh_stage = op_.tile([128, N], f32, tag="shst", name="shst")
                nc.scalar.activation(sh_stage[:], shat[sb][:], AF.Identity, bias=zcol_c)
                nc.sync.dma_start(shat_d[128 * sb: 128 * (sb + 1), :], sh_stage[:])

            rcp2 = cp.tile([128, NT], f32, tag="rcp2", name="rcp2")
            nc.vector.reciprocal(rcp2[:], z2[:])
            for sb in range(NT):
                slp = op_.tile([128, N], f32, tag="slp", name="slp")
                nc.vector.tensor_scalar_mul(
                    slp[:], Eo4[:, N * sb: N * (sb + 1)], rcp2[:, sb: sb + 1]
                )
                nc.sync.dma_start(slp_d[128 * sb: 128 * (sb + 1), :], slp[:])
                sq = op_.tile([128, N], f32, tag="sq", name="sq")
                nc.scalar.activation(
                    sq[:],
                    slp[:],
                    AF.Square,
                    bias=neginvN,
                    accum_out=msum[:, sb: sb + 1],
                )
            softc = cp.tile([128, NT], f32, tag="softc", name="softc")
            nc.scalar.activation(softc[:], msum[:], AF.Sqrt, bias=zcol_c, scale=1.0 / (N - 1))
            nc.sync.dma_start(
                soft_d.ap().rearrange("(s p) -> p s", p=128), softc[:]
            )

    nc.compile()
    return nc


# --------------------------------------------------------------------------
# host driver
# --------------------------------------------------------------------------


def _make_in_maps(
    x_s, edge_index_s, batch_s, pos_world_s,
    x_t, edge_index_t, batch_t, pos_world_t,
    W1_self, W1_nbr, b1, W2_self, W2_nbr, b2,
    Wm1, bm1, Wm2, bm2,
):
    f = np.float32
    x_s = np.asarray(x_s, f)
    x_t = np.asarray(x_t, f)
    ps = np.asarray(pos_world_s, f).reshape(B, N, 3)
    pt = np.asarray(pos_world_t, f).reshape(B, N, 3)

    wblob = _build_wblob(
        np.asarray(W1_self, f), np.asarray(W1_nbr, f), np.asarray(b1, f),
        np.asarray(W2_self, f), np.asarray(W2_nbr, f), np.asarray(b2, f),
        np.asarray(Wm1, f), np.asarray(bm1, f), np.asarray(Wm2, f),
        np.asarray(bm2, f),
    )
    wbank = _build_wbank(np.asarray(Wm2, f))
    rs = _host_random(2)

    in_maps = []
    for g in range(NCORES):
        xs_g = x_s[g * N:(g + 1) * N]
        xt_g = x_t[g * N:(g + 1) * N]
        xblob = np.empty((128, 1536), f)
        xblob[:, 0:256] = _tile4(xs_g)
        xblob[:, 256:512] = _tile4(xt_g)
        xblob[0:64, 512:1024] = xs_g.T
        xblob[64:128, 512:1024] = 0.0
        xblob[0:64, 1024:1536] = xt_g.T
        xblob[64:128, 1024:1536] = 0.0

        ablob = np.empty((128, 4096), f)
        ablob[:, 0:2048] = _tile4(_adjT(edge_index_s, g))
        ablob[:, 2048:4096] = _tile4(_adjT(edge_index_t, g))

        rblob = np.zeros((128, 1280), f)
        rblob[:, 0:128] = _tile4(rs[0][g])
        rblob[:, 128:256] = _tile4(rs[1][g])
        rblob[0:32, 256:768] = rs[0][g].T
        rblob[0:32, 768:1280] = rs[1][g].T

        d = ps[g][:, None, :] - pt[g][None, :, :]
        dist = np.sqrt((d * d).sum(-1, dtype=f).astype(f)).astype(f)
        tmat = (f(1.0) / dist).astype(f)
        tblob = _tile4(tmat)

        in_maps.append(
            {
                "wblob": wblob,
                "xblob": xblob,
                "ablob": ablob,
                "rblob": rblob,
                "wbank": wbank,
                "tblob": tblob,
            }
        )
    return in_maps


def run(in_maps, trace=False, **kw):
    from concourse.bass_utils import run_bass_kernel_spmd

    nc = _build_nc()
    return run_bass_kernel_spmd(
        nc, in_maps, list(range(NCORES)), trace=trace, **kw
    )


def _assemble(results):
    shat = np.stack([r["shat_out"] for r in results]).astype(np.float32)
    slp = np.concatenate([r["slp_out"] for r in results]).astype(np.float32)
    soft = np.concatenate([r["soft_out"] for r in results]).astype(np.float32)
    return shat, slp, soft


def kernel(**inputs):
    in_maps = _make_in_maps(**inputs)
    res = run(in_maps)
    return _assemble(res.results)


# revision 10
# speedup vs baseline: 1.0386x; 1.0386x over previous
"""DGMC graph-matching kernel for 8 Trainium2 NeuronCores.

Strategy (data-parallel over the B=8 graph dimension, one graph per core):

 * Every matmul is phrased in "transposed" (feature-major) form so that no
   on-device transposes are needed; the host supplies pre-transposed copies
   of the small inputs packed into a few large [128, X] blobs (one DMA each).
 * Edge lists are converted on the host into dense per-graph count matrices
   A^T[n, d] (#edges n->d), so GNN sum-aggregation becomes plain matmuls.
 * The expensive [Ns, Nt, R] pairwise-difference MLP never materializes:
   relu((o_s[s]-o_t[t]) @ Wm1 + bm1) @ Wm2 distributes over the difference,
   so with A_s = o_s@Wm1, A_t = o_t@Wm1 ([512, 32] each) the update is
       d[s, t] = sum_r Wm2[r] * relu(A_s[s, r] - A_t[t, r] + bm1[r]) + bm2.
   Pairwise tiles X[(j, r), t] = relu(A_s[4*jj+j, r] - A_t[t, r] + bm1[r])
   (partition = 4 s-values x 32 channels) are generated by fused
   add+relu ops split across the Vector and Scalar engines, and reduced over
   the 32 channels by PE matmuls with shifted block-diagonal Wm2 weights that
   accumulate directly into a persistent PSUM copy of S_hat.
 * Softmax normalization is folded into a row-scaling of r_s (the normalized
   attention matrix is never materialized).
 * T = 1/dist is pure input preprocessing -> computed on host, DMA'd in.
"""

import numpy as np
from functools import lru_cache

B, N, C, R = 8, 512, 64, 32
NT = N // 128  # 4 partition-tiles per 512
NCORES = 8

# matmul sites that run in float32 instead of float32r (higher precision,
# 4x slower per row).  float32r is ~1 cycle/row at N>=512.
F32_SITES = {"shat0", "psi1_agg", "psi1_h"}

# fraction of X tiles produced on the scalar engine (rest on vector engine)
ACT_EVERY = 3  # k % ACT_EVERY == 2 -> ACT
STEPS = 2  # debug knob


# --------------------------------------------------------------------------
# host-side packing
# --------------------------------------------------------------------------

# wblob column layout (partition rows x free cols), all f32:
#   W1s   [64, 64]   @ (0,    0)
#   W1n   [64, 64]   @ (0,   64)
#   W2s   [32, 32]   @ (0,  128)
#   W2n   [32, 32]   @ (0,  160)
#   negWm1x4 [32,128] @ (0,  192)
#   Wm1bank [32,512] @ (0,  320)
#   b1c   [64, 1]    @ (0,  832)
#   b2c   [32, 1]    @ (0,  833)
#   bm1x4 [128, 1]   @ (0,  834)
#   ones1 [1, 128]   @ (0,  840)
#   bm2row [1, 512]  @ (0,  968)
WBLOB_F = 1480


def _build_wblob(W1_self, W1_nbr, b1, W2_self, W2_nbr, b2, Wm1, bm1, Wm2, bm2):
    wb = np.zeros((128, WBLOB_F), np.float32)
    wb[0:64, 0:64] = W1_self
    wb[0:64, 64:128] = W1_nbr
    wb[0:32, 128:160] = W2_self
    wb[0:32, 160:192] = W2_nbr
    wb[0:32, 192:320] = np.tile(-Wm1, (1, 4))
    for j in range(4):
        wb[0:32, 320 + 128 * j + 32 * j: 320 + 128 * j + 32 * (j + 1)] = Wm1
    wb[0:64, 832] = b1
    wb[0:32, 833] = b2
    wb[0:128, 834] = np.tile(bm1, 4)
    wb[0:1, 840:968] = 1.0
    wb[0:128, 836] = -1.0 / 512.0
    wb[0:1, 968:1480] = float(bm2[0])
    return wb


def _build_wbank(Wm2):
    # 32 shifted block-diagonal lhsT blocks of [128, 128]:
    #   wbank[32*j + r, 128*k + m] = Wm2[r] iff m == 4*k + j
    w2 = Wm2[:, 0].astype(np.float32)
    bank = np.zeros((128, 128 * 32), np.float32)
    for k in range(32):
        for j in range(4):
            bank[32 * j: 32 * (j + 1), 128 * k + 4 * k + j] = w2
    return bank


def _tile4(x):  # [512, F] -> [128, 4*F] (partition-tiles side by side)
    F = x.shape[1]
    out = np.empty((128, 4 * F), np.float32)
    for b in range(4):
        out[:, F * b: F * (b + 1)] = x[128 * b: 128 * (b + 1)]
    return out


def _adjT(edge_index, g):
    # A^T[n, d] = #edges with src n -> dst d  (graph-local indices)
    src = np.asarray(edge_index[0], np.int64)
    dst = np.asarray(edge_index[1], np.int64)
    lo, hi = g * N, (g + 1) * N
    m = (src >= lo) & (src < hi)
    a = np.zeros((N, N), np.float32)
    np.add.at(a, (src[m] - lo, dst[m] - lo), 1.0)
    return a


def _host_random(num_steps):
    # Must exactly mirror the reference: jax.random on the *default* device
    # (the PRNG impl here is rbg, whose bits are platform-dependent).
    import jax

    rkey = jax.random.key(42)
    return [
        np.asarray(
            jax.random.normal(jax.random.fold_in(rkey, step), (B, N, R), "float32")
        )
        for step in range(num_steps)
    ]


# --------------------------------------------------------------------------
# the Bass program (identical for all cores)
# --------------------------------------------------------------------------


@lru_cache(maxsize=1)
def _build_nc():
    import concourse.bacc as bacc
    import concourse.bass as bass
    import concourse.tile as tile
    from concourse import mybir

    dt = mybir.dt
    f32 = dt.float32
    f32r = dt.float32r
    AF = mybir.ActivationFunctionType
    ALU = mybir.AluOpType
    AX = mybir.AxisListType

    nc = bacc.Bacc("TRN2", target_bir_lowering=False, debug=False)

    wblob_d = nc.dram_tensor("wblob", [128, WBLOB_F], f32, kind="ExternalInput")
    xblob_d = nc.dram_tensor("xblob", [128, 1536], f32, kind="ExternalInput")
    ablob_d = nc.dram_tensor("ablob", [128, 4096], f32, kind="ExternalInput")
    rblob_d = nc.dram_tensor("rblob", [128, 1280], f32, kind="ExternalInput")
    wbank_d = nc.dram_tensor("wbank", [128, 4096], f32, kind="ExternalInput")
    tblob_d = nc.dram_tensor("tblob", [128, 2048], f32, kind="ExternalInput")

    shat_d = nc.dram_tensor("shat_out", [N, N], f32, kind="ExternalOutput")
    slp_d = nc.dram_tensor("slp_out", [N, N], f32, kind="ExternalOutput")
    soft_d = nc.dram_tensor("soft_out", [N], f32, kind="ExternalOutput")

    def mm(out, lhsT, rhs, site, **kw):
        t = f32 if site in F32_SITES else f32r
        nc.tensor.matmul(out, lhsT.bitcast(t), rhs.bitcast(t), **kw)

    with tile.TileContext(nc) as tc:
        with (
            tc.tile_pool(name="cp", bufs=1) as cp,
            tc.tile_pool(name="xp", bufs=8) as xp,
            tc.tile_pool(name="op", bufs=2) as op_,
            tc.tile_pool(name="shp", bufs=1, space=bass.MemorySpace.PSUM) as shp,
            tc.tile_pool(name="rp", bufs=1, space=bass.MemorySpace.PSUM) as rp,
            tc.tile_pool(name="wp", bufs=3, space=bass.MemorySpace.PSUM) as wp,
        ):
            # ---------------- persistent SBUF tiles + input DMAs
            # (issue order = need order: psi1 first, s-side next, T last)
            wblob = cp.tile([128, WBLOB_F], f32, tag="wblob", name="wblob")
            wblob_r = cp.tile([128, WBLOB_F], f32r, tag="wblob_r", name="wblob_r")
            xblob = cp.tile([128, 1536], f32, tag="xblob", name="xblob")
            ablob_f = cp.tile([128, 4096], f32, tag="ablob_f", name="ablob_f")
            ablob = cp.tile([128, 4096], f32r, tag="ablob", name="ablob")
            rblob = cp.tile([128, 1280], f32r, tag="rblob", name="rblob")
            wbank = cp.tile([128, 4096], f32r, tag="wbank", name="wbank")
            tblob = cp.tile([128, 2048], f32, tag="tblob", name="tblob")
            nc.sync.dma_start(wblob[:], wblob_d[:])
            nc.sync.dma_start(xblob[:], xblob_d[:])
            nc.sync.dma_start(ablob_f[:, 0:2048], ablob_d[0:128, 0:2048])
            nc.sync.dma_start(ablob_f[:, 2048:4096], ablob_d[0:128, 2048:4096])
            nc.sync.dma_start(rblob[:], rblob_d.ap().bitcast(f32r))
            nc.sync.dma_start(wblob_r[:], wblob_d.ap().bitcast(f32r))
            nc.sync.dma_start(ablob[:], ablob_d.ap().bitcast(f32r))
            nc.sync.dma_start(wbank[:], wbank_d.ap().bitcast(f32r))
            nc.sync.dma_start(tblob[:], tblob_d[:])

            W1s = wblob[0:64, 0:64]
            W1n = wblob[0:64, 64:128]
            W2s = wblob_r[0:32, 128:160]
            W2n = wblob_r[0:32, 160:192]
            negWm1x4 = wblob_r[0:32, 192:320]
            Wm1bank = wblob_r[0:32, 320:832]
            b1c = wblob[0:64, 832:833]
            b2c = wblob[0:32, 833:834]
            bm1x4 = wblob[0:128, 834:835]
            zcol_c = wblob[0:128, 835:836]
            neginvN = wblob[0:128, 836:837]
            ones1 = wblob_r[0:1, 840:968]
            bm2row = wblob_r[0:1, 968:1480]

            xs4 = xblob[:, 0:256]
            xt4 = xblob[:, 256:512]
            xsT = xblob[0:64, 512:1024]
            xtT = xblob[0:64, 1024:1536]
            AsT4 = ablob[:, 0:2048]
            AtT4 = ablob[:, 2048:4096]
            rs4 = [rblob[:, 0:128], rblob[:, 128:256]]
            rsT = [rblob[0:32, 256:768], rblob[0:32, 768:1280]]

            # persistent PSUM: S_hat (4 banks) + repneg (1 bank)
            shat = [shp.tile([128, N], f32, tag=f"shat{sb}", name=f"shat{sb}")
                    for sb in range(NT)]

            # ---------------- psi_1 + S_hat_0 (f32 throughout)
            hT = {}
            for side, x4, xT, AT4f in (
                ("s", xs4, xsT, ablob_f[:, 0:2048]),
                ("t", xt4, xtT, ablob_f[:, 2048:4096]),
            ):
                agg_ps = wp.tile([64, N], f32, tag="w", name="w")
                for nb in range(NT):
                    mm(agg_ps[:], x4[:, 64 * nb: 64 * (nb + 1)],
                       AT4f[:, N * nb: N * (nb + 1)], "psi1_agg",
                       start=(nb == 0), stop=(nb == NT - 1))
                agg_sb = cp.tile([64, N], f32, tag=f"agg1{side}", name=f"agg1{side}")
                nc.scalar.activation(agg_sb[:], agg_ps[:], AF.Identity,
                                     bias=zcol_c[0:64])
                h_ps = wp.tile([64, N], f32, tag="w", name="w")
                mm(h_ps[:], W1s, xT, "psi1_h", start=True, stop=False)
                mm(h_ps[:], W1n, agg_sb[:], "psi1_h", start=False, stop=True)
                hT[side] = cp.tile([64, N], f32, tag=f"hT{side}", name=f"hT{side}")
                nc.scalar.activation(hT[side][:], h_ps[:], AF.Relu, bias=b1c)

            for sb in range(NT):
                mm(shat[sb][:], hT["s"][:, 128 * sb: 128 * (sb + 1)], hT["t"][:],
                   "shat0", start=True, stop=(STEPS == 0), skip_group_check=True)

            # ---------------- s-side precompute (softmax-independent), both steps
            biasbs = []
            for step in range(STEPS):
                agg_ps = wp.tile([32, N], f32, tag="w", name="w")
                for nb in range(NT):
                    mm(agg_ps[:], rs4[step][:, R * nb: R * (nb + 1)],
                       AsT4[:, N * nb: N * (nb + 1)], "agg2",
                       start=(nb == 0), stop=(nb == NT - 1))
                agg_sb = cp.tile([32, N], f32r, tag=f"agg2s{step}",
                                 name=f"agg2s{step}")
                nc.scalar.activation(agg_sb[:], agg_ps[:], AF.Identity,
                                     bias=zcol_c[0:32])
                o_ps = wp.tile([32, N], f32, tag="w", name="w")
                mm(o_ps[:], W2s, rsT[step], "o2", start=True, stop=False)
                mm(o_ps[:], W2n, agg_sb[:], "o2", start=False, stop=True)
                osT = cp.tile([32, N], f32r, tag=f"osT{step}", name=f"osT{step}")
                nc.scalar.activation(osT[:], o_ps[:], AF.Relu, bias=b2c)
                biasb_ps = wp.tile([128, 128], f32, tag="w", name="w")
                for j in range(4):
                    mm(biasb_ps[:], Wm1bank[:, 128 * j: 128 * (j + 1)],
                       osT[:, j::4], "amlp", start=(j == 0), stop=(j == 3))
                biasb = cp.tile([128, 128], f32, tag=f"biasb{step}",
                                name=f"biasb{step}")
                nc.scalar.activation(biasb[:], biasb_ps[:], AF.Identity, bias=bm1x4)
                biasbs.append(biasb)

            # ---------------- per-step softmax tiles (step 0 standalone;
            # step >=1 interleaved into the previous X phase)
            negmaxs = [cp.tile([128, NT], f32, tag=f"nm{s}", name=f"nm{s}")
                       for s in range(STEPS)]
            zcols = [cp.tile([128, NT], f32, tag=f"zc{s}", name=f"zc{s}")
                     for s in range(STEPS)]
            rcps = [cp.tile([128, NT], f32, tag=f"rc{s}", name=f"rc{s}")
                    for s in range(STEPS)]
            E4s = [cp.tile([128, NT * N], f32r, tag=f"E4_{s}", name=f"E4_{s}")
                   for s in range(STEPS)]
            rs_scs = [cp.tile([128, NT * R], f32r, tag=f"rsc{s}", name=f"rsc{s}")
                      for s in range(STEPS)]

            def softmax_tiles(step, sb):
                nc.vector.tensor_reduce(negmaxs[step][:, sb: sb + 1], shat[sb][:],
                                        AX.X, ALU.max, negate=True)
                nc.scalar.activation(E4s[step][:, N * sb: N * (sb + 1)], shat[sb][:],
                                     AF.Exp, bias=negmaxs[step][:, sb: sb + 1],
                                     accum_out=zcols[step][:, sb: sb + 1])
                nc.vector.reciprocal(rcps[step][:, sb: sb + 1],
                                     zcols[step][:, sb: sb + 1])
                nc.vector.tensor_scalar_mul(
                    rs_scs[step][:, R * sb: R * (sb + 1)],
                    rs4[step][:, R * sb: R * (sb + 1)].bitcast(f32),
                    rcps[step][:, sb: sb + 1])

            # final-phase tiles (interleaved into the last X phase)
            negmax2 = cp.tile([128, NT], f32, tag="negmax2", name="negmax2")
            z2 = cp.tile([128, NT], f32, tag="z2", name="z2")
            rcp2 = cp.tile([128, NT], f32, tag="rcp2", name="rcp2")
            Eo4 = cp.tile([128, NT * N], f32, tag="Eo4", name="Eo4")
            msum = cp.tile([128, NT], f32, tag="msum", name="msum")

            def final_tiles(sb):
                Zt = op_.tile([128, N], f32, tag="zt", name="zt")
                nc.vector.tensor_tensor(Zt[:], shat[sb][:],
                                        tblob[:, N * sb: N * (sb + 1)], ALU.add)
                nc.vector.tensor_reduce(negmax2[:, sb: sb + 1], Zt[:], AX.X,
                                        ALU.max, negate=True)
                nc.scalar.activation(Eo4[:, N * sb: N * (sb + 1)], Zt[:], AF.Exp,
                                     bias=negmax2[:, sb: sb + 1],
                                     accum_out=z2[:, sb: sb + 1])
                nc.vector.reciprocal(rcp2[:, sb: sb + 1], z2[:, sb: sb + 1])
                slp = op_.tile([128, N], f32, tag="slp", name="slp")
                nc.vector.tensor_scalar_mul(slp[:], Eo4[:, N * sb: N * (sb + 1)],
                                            rcp2[:, sb: sb + 1])
                nc.sync.dma_start(slp_d[128 * sb: 128 * (sb + 1), :], slp[:])
                sq = op_.tile([128, N], f32, tag="sq", name="sq")
                nc.scalar.activation(sq[:], slp[:], AF.Square, bias=neginvN,
                                     accum_out=msum[:, sb: sb + 1])
                sh_stage = op_.tile([128, N], f32, tag="shst", name="shst")
                nc.scalar.activation(sh_stage[:], shat[sb][:], AF.Identity,
                                     bias=zcol_c)
                nc.sync.dma_start(shat_d[128 * sb: 128 * (sb + 1), :], sh_stage[:])

            if STEPS > 0:
                for sb in range(NT):
                    softmax_tiles(0, sb)

            # ---------------- consensus steps
            for step in range(STEPS):
                # t-side chain (depends on this step's softmax)
                rtT_ps = wp.tile([32, N], f32, tag="w", name="w")
                for sb in range(NT):
                    mm(rtT_ps[:], rs_scs[step][:, R * sb: R * (sb + 1)],
                       E4s[step][:, N * sb: N * (sb + 1)], "rtT",
                       start=(sb == 0), stop=(sb == NT - 1))
                rtT = cp.tile([32, N], f32r, tag="rtT", name="rtT")
                nc.scalar.activation(rtT[:], rtT_ps[:], AF.Identity,
                                     bias=zcol_c[0:32])
                rt4 = cp.tile([128, NT * R], f32r, tag="rt4", name="rt4")
                for tb in range(NT):
                    rt_ps = wp.tile([128, R], f32, tag="w", name="w")
                    for sb in range(NT):
                        mm(rt_ps[:],
                           E4s[step][:, N * sb + 128 * tb: N * sb + 128 * (tb + 1)],
                           rs_scs[step][:, R * sb: R * (sb + 1)], "rtnat",
                           start=(sb == 0), stop=(sb == NT - 1))
                    nc.vector.tensor_copy(rt4[:, R * tb: R * (tb + 1)], rt_ps[:])
                agg_ps = wp.tile([32, N], f32, tag="w", name="w")
                for nb in range(NT):
                    mm(agg_ps[:], rt4[:, R * nb: R * (nb + 1)],
                       AtT4[:, N * nb: N * (nb + 1)], "agg2",
                       start=(nb == 0), stop=(nb == NT - 1))
                agg_tb = cp.tile([32, N], f32r, tag="agg2t", name="agg2t")
                nc.scalar.activation(agg_tb[:], agg_ps[:], AF.Identity,
                                     bias=zcol_c[0:32])
                o_ps = wp.tile([32, N], f32, tag="w", name="w")
                mm(o_ps[:], W2s, rtT[:], "o2", start=True, stop=False)
                mm(o_ps[:], W2n, agg_tb[:], "o2", start=False, stop=True)
                otT = cp.tile([32, N], f32r, tag="otT", name="otT")
                nc.scalar.activation(otT[:], o_ps[:], AF.Relu, bias=b2c)

                repneg_ps = rp.tile([128, N], f32, tag="repneg", name="repneg_ps")
                mm(repneg_ps[:], negWm1x4, otT[:], "amlp", start=True, stop=True)
                repneg = cp.tile([128, N], f32, tag="repnegs", name="repnegs")
                nc.scalar.activation(repneg[:], repneg_ps[:], AF.Identity,
                                     bias=zcol_c)

                # X phase with interleaved next-phase tiles
                biasb = biasbs[step]
                for sb in range(NT):
                    for k in range(32):
                        if k == 8 and sb > 0:
                            if step + 1 < STEPS:
                                softmax_tiles(step + 1, sb - 1)
                            elif step == STEPS - 1:
                                final_tiles(sb - 1)
                        jj = 32 * sb + k
                        X = xp.tile([128, N], f32r, tag="x", name="x")
                        if k % ACT_EVERY == ACT_EVERY - 1:
                            nc.scalar.activation(X[:], repneg_ps[:], AF.Relu,
                                                 bias=biasb[:, jj: jj + 1])
                        else:
                            nc.vector.tensor_scalar(X[:], repneg[:],
                                                    biasb[:, jj: jj + 1], 0.0,
                                                    ALU.add, ALU.max)
                        mm(shat[sb][:], wbank[:, 128 * k: 128 * (k + 1)], X[:],
                           "xw", start=False, stop=False, skip_group_check=True)
                    mm(shat[sb][:], ones1, bm2row, "bm2", start=False,
                       stop=(step == STEPS - 1), skip_group_check=True)
                if step + 1 < STEPS:
                    softmax_tiles(step + 1, NT - 1)
                elif step == STEPS - 1:
                    final_tiles(NT - 1)

            # ---------------- tail: std-dev sqrt + output
            softc = cp.tile([128, NT], f32, tag="softc", name="softc")
            nc.scalar.activation(softc[:], msum[:], AF.Sqrt, bias=zcol_c,
                                 scale=1.0 / (N - 1))
            nc.sync.dma_start(soft_d.ap().rearrange("(s p) -> p s", p=128),
                              softc[:])

    nc.compile()
    return nc


# --------------------------------------------------------------------------
# host driver
# --------------------------------------------------------------------------


def _make_in_maps(
    x_s, edge_index_s, batch_s, pos_world_s,
    x_t, edge_index_t, batch_t, pos_world_t,
    W1_self, W1_nbr, b1, W2_self, W2_nbr, b2,
    Wm1, bm1, Wm2, bm2,
):
    f = np.float32
    x_s = np.asarray(x_s, f)
    x_t = np.asarray(x_t, f)
    ps = np.asarray(pos_world_s, f).reshape(B, N, 3)
    pt = np.asarray(pos_world_t, f).reshape(B, N, 3)

    wblob = _build_wblob(
        np.asarray(W1_self, f), np.asarray(W1_nbr, f), np.asarray(b1, f),
        np.asarray(W2_self, f), np.asarray(W2_nbr, f), np.asarray(b2, f),
        np.asarray(Wm1, f), np.asarray(bm1, f), np.asarray(Wm2, f),
        np.asarray(bm2, f),
    )
    wbank = _build_wbank(np.asarray(Wm2, f))
    rs = _host_random(2)

    in_maps = []
    for g in range(NCORES):
        xs_g = x_s[g * N:(g + 1) * N]
        xt_g = x_t[g * N:(g + 1) * N]
        xblob = np.empty((128, 1536), f)
        xblob[:, 0:256] = _tile4(xs_g)
        xblob[:, 256:512] = _tile4(xt_g)
        xblob[0:64, 512:1024] = xs_g.T
        xblob[64:128, 512:1024] = 0.0
        xblob[0:64, 1024:1536] = xt_g.T
        xblob[64:128, 1024:1536] = 0.0

        ablob = np.empty((128, 4096), f)
        ablob[:, 0:2048] = _tile4(_adjT(edge_index_s, g))
        ablob[:, 2048:4096] = _tile4(_adjT(edge_index_t, g))

        rblob = np.zeros((128, 1280), f)
        rblob[:, 0:128] = _tile4(rs[0][g])
        rblob[:, 128:256] = _tile4(rs[1][g])
        rblob[0:32, 256:768] = rs[0][g].T
        rblob[0:32, 768:1280] = rs[1][g].T

        d = ps[g][:, None, :] - pt[g][None, :, :]
        dist = np.sqrt((d * d).sum(-1, dtype=f).astype(f)).astype(f)
        tmat = (f(1.0) / dist).astype(f)
        tblob = _tile4(tmat)

        in_maps.append(
            {
                "wblob": wblob,
                "xblob": xblob,
                "ablob": ablob,
                "rblob": rblob,
                "wbank": wbank,
                "tblob": tblob,
            }
        )
    return in_maps


def run(in_maps, trace=False, **kw):
    from concourse.bass_utils import run_bass_kernel_spmd

    nc = _build_nc()
    return run_bass_kernel_spmd(
        nc, in_maps, list(range(NCORES)), trace=trace, **kw
    )


def _assemble(results):
    shat = np.stack([r["shat_out"] for r in results]).astype(np.float32)
    slp = np.concatenate([r["slp_out"] for r in results]).astype(np.float32)
    soft = np.concatenate([r["soft_out"] for r in results]).astype(np.float32)
    return shat, slp, soft


def kernel(**inputs):
    in_maps = _make_in_maps(**inputs)
    res = run(in_maps)
    return _assemble(res.results)
